# revision 18
# baseline (speedup 1.0000x reference)
"""BitnetMLP on 8 TRN2 NeuronCores — Megatron tensor-parallel over the
intermediate dim I, exact integer matmuls, host-side quantization.

Host precomputes (exact f32 math, matching the reference's fake-quant):
  - per-token int8 activation quant of x  -> qxT int values stored as bf16
    (exact), per-token dequant scale vectors bmc = mc*m_gate, mcu = mc*m_up
  - per-tensor ternary weight quant       -> wg/wu/wd int values as fp8e4
    (exact), global dequant scalars m_*
so the device runs a pure matmul pipeline: no weight-stats pass, no x-quant
prepass, no weight-quant pass.

Device per core r (shards I: columns of w_gate/w_up, rows of w_down):
  phase1(g): gate/up int matmuls (PSUM f32 exact), silu*up -> h (f32),
             h^2 partial sums (PE) + |lnw*h| partial max -> AllGather
  phase2a(g): token stats: var -> rsqrt (Newton), h-quant scale f_t,
              down dequant scale d_t
  phase2q(g): int8 quant of h (stored bf16)
  phase2d(g): down int matmuls, per-token dequant, ReduceScatter(add)
              straight into the bf16 output
Phases of group g-1 are interleaved into phase1(g)'s emission so the PE
stream stays dense and the AllGather latency is hidden.
"""
import numpy as np

N_CORES = 8
B, S, H, I = 2, 2048, 2048, 8192
T = B * S                      # 4096 tokens
ISH = I // N_CORES             # 1024 I-shard per core
TG = 512                       # tokens per group
NG = T // TG                   # 8 groups
KC = H // 128                  # 16 contract chunks for gate/up
IC = ISH // 128                # 8 contract chunks for down
NTC = TG // 128                # 4 token tiles per group
NH = 2048 // 512               # 4 output col groups for down
RPG = TG // N_CORES            # 64 output rows per group per core

MAGIC = float(1.5 * 2 ** 23)   # f32 round-to-nearest-even forcing constant
EPS = 1e-5
RMS_EPS = 1e-6

_CACHED = {}


def _build():
    import concourse.bass as bass
    import concourse.bacc as bacc
    import concourse.tile as tile
    import concourse.mybir as mybir
    from concourse import masks
    from contextlib import ExitStack

    dt = mybir.dt
    AO = mybir.AluOpType
    AF = mybir.ActivationFunctionType
    RG = [list(range(N_CORES))]

    nc = bacc.Bacc("TRN2", target_bir_lowering=False, debug=False,
                   num_devices=N_CORES)

    qxT_in = nc.dram_tensor("qxT", [H, T], dt.bfloat16, kind="ExternalInput")
    wg_in = nc.dram_tensor("wg", [H, ISH], dt.float8e4, kind="ExternalInput")
    wu_in = nc.dram_tensor("wu", [H, ISH], dt.float8e4, kind="ExternalInput")
    wd_in = nc.dram_tensor("wd", [ISH, 2048], dt.float8e4, kind="ExternalInput")
    lnw_in = nc.dram_tensor("lnw", [ISH], dt.float32, kind="ExternalInput")
    bmc_in = nc.dram_tensor("bmc", [T], dt.float32, kind="ExternalInput")
    mcu_in = nc.dram_tensor("mcu", [T], dt.float32, kind="ExternalInput")
    meta_in = nc.dram_tensor("meta", [8], dt.float32, kind="ExternalInput")
    y_out = nc.dram_tensor("y_out", [T // N_CORES, 2048], dt.bfloat16,
                           kind="ExternalOutput")

    with tile.TileContext(nc) as tc:
        with ExitStack() as stack:
            ep = stack.enter_context
            constp = ep(tc.tile_pool(name="const", bufs=1))
            wgp = ep(tc.tile_pool(name="wgp", bufs=1))
            qxp = ep(tc.tile_pool(name="qx", bufs=3))
            bcp = ep(tc.tile_pool(name="bc", bufs=3))
            fcp = ep(tc.tile_pool(name="fc", bufs=2))
            hp = ep(tc.tile_pool(name="hbuf", bufs=2))
            qhp = ep(tc.tile_pool(name="qh", bufs=3))
            dtp = ep(tc.tile_pool(name="dtp", bufs=3))
            evp = ep(tc.tile_pool(name="evac", bufs=2))
            mxp = ep(tc.tile_pool(name="mx", bufs=2))
            stp = ep(tc.tile_pool(name="stats", bufs=2))
            rowp = ep(tc.tile_pool(name="rows", bufs=2))
            yrp = ep(tc.tile_pool(name="yrow", bufs=2))
            ps_gu = ep(tc.tile_pool(name="ps_gu", bufs=4, space="PSUM"))
            ps_dn = ep(tc.tile_pool(name="ps_dn", bufs=2, space="PSUM"))
            ps_ss = ep(tc.tile_pool(name="ps_ss", bufs=1, space="PSUM"))
            ps_misc = ep(tc.tile_pool(name="ps_misc", bufs=1, space="PSUM"))
            dram = ep(tc.tile_pool(name="dram", bufs=1, space="DRAM"))
            dram_rs = ep(tc.tile_pool(name="dram_rs", bufs=8, space="DRAM"))

            # ---------- constants ----------
            ident = constp.tile([128, 128], dt.float32)
            masks.make_identity(nc, ident[:])
            ones_col_bf = constp.tile([128, 1], dt.bfloat16)
            nc.vector.memset(ones_col_bf[:], 1.0)
            lnw_sb = constp.tile([128, IC], dt.float32)
            nc.sync.dma_start(lnw_sb[:], lnw_in.rearrange("(c p) -> p c", p=128)[:])
            alnw_sb = constp.tile([128, IC], dt.float32)
            nc.vector.tensor_scalar(alnw_sb.bitcast(dt.uint32)[:],
                                    lnw_sb.bitcast(dt.uint32)[:],
                                    0x7FFFFFFF, None, AO.bitwise_and)
            md_b = constp.tile([32, 1], dt.float32)
            nc.sync.dma_start(md_b[:], meta_in[0:1]
                              .rearrange("(o f) -> o f", o=1)
                              .partition_broadcast(32))

            # ---------- weights (fp8 ternary, host-prepared) ----------
            # gate/up: 4 tiles each of [128, 4(kc), ISH]; down: 2 of [128,4,2048]
            # DMA triggers spread across engine queues so blocks land in
            # parallel and matmuls can begin ASAP (emitted after prefetch(0)).
            wg_t, wu_t, wd_t = [], [], []
            weng = [nc.sync, nc.gpsimd, nc.scalar]

            def emit_weight_dmas():
                for bidx in range(4):
                    tg = wgp.tile([128, 4 * ISH], dt.float8e4, tag=f"wg{bidx}")
                    weng[(2 * bidx) % 3].dma_start(
                        tg.rearrange("p (k i) -> p k i", k=4)[:],
                        wg_in[bidx * 512:(bidx + 1) * 512, :]
                        .rearrange("(k p) i -> p k i", p=128)[:])
                    wg_t.append(tg.rearrange("p (k i) -> p k i", k=4))
                    tu = wgp.tile([128, 4 * ISH], dt.float8e4, tag=f"wu{bidx}")
                    weng[(2 * bidx + 1) % 3].dma_start(
                        tu.rearrange("p (k i) -> p k i", k=4)[:],
                        wu_in[bidx * 512:(bidx + 1) * 512, :]
                        .rearrange("(k p) i -> p k i", p=128)[:])
                    wu_t.append(tu.rearrange("p (k i) -> p k i", k=4))
                for bidx in range(2):
                    td = wgp.tile([128, 4 * 2048], dt.float8e4, tag=f"wd{bidx}")
                    weng[bidx].dma_start(
                        td.rearrange("p (k i) -> p k i", k=4)[:],
                        wd_in[bidx * 512:(bidx + 1) * 512, :]
                        .rearrange("(k p) i -> p k i", p=128)[:])
                    wd_t.append(td.rearrange("p (k i) -> p k i", k=4))

            # ---------- internal DRAM ----------
            y_partial = dram.tile([T, 2048], dt.bfloat16)
            stat_in = dram.tile([NG, 2, TG], dt.float32)
            stat_out = dram.tile([NG, 2 * N_CORES, TG], dt.float32)
            bounce = dram.tile([NG, 2, TG], dt.float32)

            # ---------- per-group state ----------
            qx_slots, bmc_slots, h_slots, qh_slots = {}, {}, {}, {}
            f_slots, dt_slots = {}, {}

            def emit_prefetch(g, nq=2):
                tok0 = g * TG
                qx = qxp.tile([128, KC * TG], dt.bfloat16, tag="qx")
                qx_slots[g] = qx
                qv = qx.rearrange("p (k t) -> p k t", k=KC)
                src = qxT_in.rearrange("(k p) t -> p k t", p=128)
                step = KC // nq
                for q in range(nq):
                    eng = (nc.sync, nc.gpsimd, nc.scalar)[q % 3]
                    eng.dma_start(qv[:, q * step:(q + 1) * step, :],
                                  src[:, q * step:(q + 1) * step,
                                      tok0:tok0 + TG])
                bmc_t = bcp.tile([128, TG], dt.float32, tag="bmc")
                bmc_slots[g] = bmc_t
                nc.sync.dma_start(bmc_t[:], bmc_in[tok0:tok0 + TG]
                                  .rearrange("(o f) -> o f", o=1)
                                  .partition_broadcast(128))

            # ---------- phase 2a: token stats from gathered partials ----------
            stat32_slots = {}

            def emit_phase2a_prep(g):
                stat32 = stp.tile([32, TG], dt.float32, tag="stat32")
                stat32_slots[g] = stat32
                nc.vector.memset(stat32[:], 0.0)
                nc.sync.dma_start(stat32[0:2 * N_CORES, :], stat_out[g])

            def emit_phase2a(g):
                tok0 = g * TG
                J = TG // 32
                stat32 = stat32_slots.pop(g)
                st32 = stp.tile([32, TG], dt.float32, tag="st32")
                nc.vector.transpose(st32[:], stat32[:])
                stv = st32.rearrange("p (j h a two) -> p j h two a",
                                     h=2, two=2, a=N_CORES)
                ssg = stp.tile([32, J], dt.float32, tag="ssg")
                nc.vector.tensor_reduce(ssg[:], stv[:, :, 0:1, 0:1, :],
                                        mybir.AxisListType.X, AO.add)
                pmg = stp.tile([32, J], dt.float32, tag="pmg")
                nc.vector.tensor_reduce(pmg[:], stv[:, :, 0:1, 1:2, :],
                                        mybir.AxisListType.X, AO.max)
                mcu32 = stp.tile([32, J], dt.float32, tag="mcu32")
                nc.sync.dma_start(mcu32[:], mcu_in[tok0:tok0 + TG]
                                  .rearrange("(j q) -> q j", q=32)[:])
                # var = ssg*mcu^2/I + eps ; rt = rsqrt(var) via newton
                mcu2 = stp.tile([32, J], dt.float32, tag="mcu2")
                nc.vector.tensor_tensor(mcu2[:], mcu32[:], mcu32[:], AO.mult)
                var = stp.tile([32, J], dt.float32, tag="var")
                nc.vector.tensor_tensor(var[:], ssg[:], mcu2[:], AO.mult)
                nc.vector.tensor_scalar(var[:], var[:], float(1.0 / I),
                                        RMS_EPS, AO.mult, AO.add)
                sq = stp.tile([32, J], dt.float32, tag="sq")
                nc.scalar.sqrt(sq[:], var[:])
                rt = stp.tile([32, J], dt.float32, tag="rt")
                nc.vector.reciprocal(rt[:], sq[:])
                ntn = stp.tile([32, J], dt.float32, tag="ntn")
                nc.vector.tensor_tensor(ntn[:], sq[:], rt[:], AO.mult)
                nc.vector.tensor_scalar(ntn[:], ntn[:], -1.0, 2.0, AO.mult, AO.add)
                nc.vector.tensor_tensor(rt[:], rt[:], ntn[:], AO.mult)
                # A = clip(pmg*mcu*rt, EPS) ; rec = 1/A (newton)
                A = stp.tile([32, J], dt.float32, tag="A")
                nc.vector.tensor_tensor(A[:], pmg[:], mcu32[:], AO.mult)
                nc.vector.tensor_tensor(A[:], A[:], rt[:], AO.mult)
                nc.vector.tensor_scalar(A[:], A[:], EPS, None, AO.max)
                rec = stp.tile([32, J], dt.float32, tag="rec")
                nc.vector.reciprocal(rec[:], A[:])
                ntn2 = stp.tile([32, J], dt.float32, tag="ntn2")
                nc.vector.tensor_tensor(ntn2[:], A[:], rec[:], AO.mult)
                nc.vector.tensor_scalar(ntn2[:], ntn2[:], -1.0, 2.0,
                                        AO.mult, AO.add)
                nc.vector.tensor_tensor(rec[:], rec[:], ntn2[:], AO.mult)
                # f = 127 * mcu * rt * rec ; d = A * md / 127
                fq = stp.tile([32, J], dt.float32, tag="fq")
                nc.vector.tensor_tensor(fq[:], mcu32[:], rt[:], AO.mult)
                nc.vector.tensor_tensor(fq[:], fq[:], rec[:], AO.mult)
                nc.vector.tensor_scalar(fq[:], fq[:], 127.0, None, AO.mult)
                d32 = stp.tile([32, J], dt.float32, tag="d32")
                nc.vector.tensor_scalar(d32[:], A[:], md_b[:, 0:1],
                                        float(1.0 / 127.0), AO.mult, AO.mult)
                nc.sync.dma_start(bounce[g, 0].rearrange("(j q) -> q j", q=32)[:],
                                  fq[:])
                f_t = fcp.tile([128, TG], dt.float32, tag="f_t")
                f_slots[g] = f_t
                nc.sync.dma_start(f_t[:], bounce[g, 0]
                                  .rearrange("(o f) -> o f", o=1)
                                  .partition_broadcast(128))
                nc.sync.dma_start(bounce[g, 1].rearrange("(j q) -> q j", q=32)[:],
                                  d32[:])
                d_t = dtp.tile([128, NTC], dt.float32, tag="d_t")
                dt_slots[g] = d_t
                nc.sync.dma_start(d_t[:], bounce[g, 1]
                                  .rearrange("(c p) -> p c", p=128)[:])

            # ---------- phase 2q: one ic chunk of h-quant ----------
            def emit_phase2q_chunk(g, ic):
                h = h_slots[g]
                qh = qh_slots[g]
                f_t = f_slots[g]
                tq = evp.tile([128, TG], dt.float32, tag="tq")
                nc.vector.scalar_tensor_tensor(
                    tq[:], h[:, ic * TG:(ic + 1) * TG], lnw_sb[:, ic:ic + 1],
                    f_t[:], AO.mult, AO.mult)
                nc.vector.tensor_scalar(tq[:], tq[:], MAGIC, -MAGIC,
                                        AO.add, AO.add)
                nc.vector.tensor_scalar(qh[:, ic * TG:(ic + 1) * TG], tq[:],
                                        127.0, -128.0, AO.min, AO.max)

            # ---------- phase 1: gate/up matmuls + h + stats ----------
            def emit_phase1(g):
                qx = qx_slots.pop(g)
                bmc_t = bmc_slots.pop(g)
                qv = qx.rearrange("p (k t) -> p k t", k=KC)
                h = hp.tile([128, IC * TG], dt.float32, tag="h")
                h_slots[g] = h
                qh = qhp.tile([128, IC * TG], dt.bfloat16, tag="qh")
                qh_slots[g] = qh
                maxt = mxp.tile([128, TG], dt.float32, tag="maxt")
                ss_ps = ps_ss.tile([1, TG], dt.float32, tag="ss_ps")
                h2_prev = [None]

                def emit_ss(ic_done):
                    nc.tensor.matmul(ss_ps[:], ones_col_bf[:], h2_prev[0][:],
                                     start=(ic_done == 0), stop=(ic_done == IC - 1))

                for ic in range(IC):
                    g_ps = ps_gu.tile([128, TG], dt.float32, tag="gu_ps")
                    for kc in range(KC):
                        nc.tensor.matmul(
                            g_ps[:],
                            wg_t[kc // 4][:, kc % 4, ic * 128:(ic + 1) * 128],
                            qv[:, kc, :], start=(kc == 0), stop=(kc == KC - 1))
                    if ic > 0:
                        emit_ss(ic - 1)
                    u_ps = ps_gu.tile([128, TG], dt.float32, tag="gu_ps")
                    for kc in range(KC):
                        nc.tensor.matmul(
                            u_ps[:],
                            wu_t[kc // 4][:, kc % 4, ic * 128:(ic + 1) * 128],
                            qv[:, kc, :], start=(kc == 0), stop=(kc == KC - 1))
                    gv = evp.tile([128, TG], dt.float32, tag="gv")
                    nc.vector.tensor_tensor(gv[:], g_ps[:], bmc_t[:], AO.mult)
                    sv = evp.tile([128, TG], dt.float32, tag="sv")
                    nc.scalar.activation(sv[:], gv[:], AF.Silu)
                    hslice = h[:, ic * TG:(ic + 1) * TG]
                    nc.vector.tensor_tensor(hslice, u_ps[:], sv[:], AO.mult)
                    h2 = evp.tile([128, TG], dt.bfloat16, tag="h2")
                    nc.gpsimd.tensor_tensor(h2[:], hslice, hslice, AO.mult)
                    h2_prev[0] = h2
                    ha = mxp.tile([128, TG], dt.float32, tag="ha")
                    nc.vector.tensor_scalar(
                        ha.bitcast(dt.uint32)[:],
                        h.bitcast(dt.uint32)[:, ic * TG:(ic + 1) * TG],
                        0x7FFFFFFF, None, AO.bitwise_and)
                    if ic == 0:
                        nc.vector.tensor_scalar(maxt[:], ha[:],
                                                alnw_sb[:, 0:1], None, AO.mult)
                    else:
                        nc.vector.scalar_tensor_tensor(
                            maxt[:], ha[:], alnw_sb[:, ic:ic + 1], maxt[:],
                            AO.mult, AO.max)
                    # interleave prior group's stats/quant into this PE stream
                    if g >= 1:
                        if ic == 0:
                            emit_phase2a_prep(g - 1)
                        elif ic == 2:
                            emit_phase2a(g - 1)
                        elif ic >= 4:
                            emit_phase2q_chunk(g - 1, 2 * (ic - 4))
                            emit_phase2q_chunk(g - 1, 2 * (ic - 4) + 1)
                emit_ss(IC - 1)
                pm_nat = rowp.tile([128, NTC], dt.float32, tag="pm_nat")
                for c in range(NTC):
                    tr_ps = ps_misc.tile([128, 512], dt.float32, tag="tr_ps")
                    nc.tensor.transpose(tr_ps[:, 0:128],
                                        maxt[:, c * 128:(c + 1) * 128], ident[:])
                    nc.vector.tensor_reduce(pm_nat[:, c:c + 1], tr_ps[:, 0:128],
                                            mybir.AxisListType.X, AO.max)
                ss_row = rowp.tile([1, TG], dt.float32, tag="ss_row")
                nc.vector.tensor_copy(ss_row[:], ss_ps[:])
                nc.scalar.dma_start(stat_in[g, 0].rearrange("(o f) -> o f", o=1)[:],
                                    ss_row[:])
                nc.scalar.dma_start(stat_in[g, 1].rearrange("(c p) -> p c", p=128)[:],
                                    pm_nat[:])
                nc.gpsimd.collective_compute(
                    "AllGather", AO.bypass, replica_groups=RG,
                    ins=[stat_in[g].opt()], outs=[stat_out[g].opt()])

            # ---------- phase 2d: down matmuls + dequant + RS ----------
            def emit_phase2d(g):
                tok0 = g * TG
                qh = qh_slots.pop(g)
                h_slots.pop(g)
                f_slots.pop(g)
                d_t = dt_slots.pop(g)
                for tcx in range(NTC):
                    y_row = yrp.tile([128, 2048], dt.bfloat16, tag="y_row")
                    for nh in range(NH):
                        y_ps = ps_dn.tile([128, 512], dt.float32, tag="y_ps")
                        for ic in range(IC):
                            nc.tensor.matmul(
                                y_ps[:],
                                qh[:, ic * TG + tcx * 128:
                                   ic * TG + (tcx + 1) * 128],
                                wd_t[ic // 4][:, ic % 4,
                                              nh * 512:(nh + 1) * 512],
                                start=(ic == 0), stop=(ic == IC - 1))
                        nc.scalar.mul(y_row[:, nh * 512:(nh + 1) * 512],
                                      y_ps[:], d_t[:, tcx:tcx + 1])
                    nc.sync.dma_start(
                        y_partial[tok0 + tcx * 128:tok0 + (tcx + 1) * 128, :],
                        y_row[:])
                rs_out = dram_rs.tile([RPG, 2048], dt.bfloat16, tag="rs_out")
                nc.gpsimd.collective_compute(
                    "ReduceScatter", AO.add, replica_groups=RG,
                    ins=[y_partial[tok0:tok0 + TG, :].opt()],
                    outs=[rs_out.opt()])
                nc.sync.dma_start(y_out[g * RPG:(g + 1) * RPG, :], rs_out[:])

            # ---------- emission ----------
            # down phase trails by TWO groups: when phase2a(g-1) is consumed
            # (inside phase1(g)) every collective emitted before it has long
            # completed — the shared cumulative CC semaphore then never stalls.
            emit_prefetch(0, nq=4)
            emit_weight_dmas()
            emit_prefetch(1)
            for g in range(NG):
                if g + 2 < NG:
                    emit_prefetch(g + 2)
                emit_phase1(g)
                if g >= 2:
                    emit_phase2d(g - 2)
            emit_phase2a_prep(NG - 1)
            emit_phase2d(NG - 2)
            emit_phase2a(NG - 1)
            for ic in range(IC):
                emit_phase2q_chunk(NG - 1, ic)
            emit_phase2d(NG - 1)

    nc.compile()
    return nc


def _get_nc():
    if "nc" not in _CACHED:
        _CACHED["nc"] = _build()
    return _CACHED["nc"]


def _tern(w):
    s = np.float32(1.0) / np.clip(np.abs(w).mean(dtype=np.float32),
                                  EPS, None).astype(np.float32)
    q = np.clip(np.rint(w * s), -1.0, 1.0).astype(np.float32)
    return q, np.float32(1.0 / s)


def _make_in_maps(x, w_gate, w_up, w_down, ln_weight):
    import ml_dtypes

    xf = np.asarray(x, dtype=np.float32).reshape(T, H)
    qg, mg = _tern(np.asarray(w_gate, dtype=np.float32))
    qu, mu = _tern(np.asarray(w_up, dtype=np.float32))
    qd, md = _tern(np.asarray(w_down, dtype=np.float32))
    lnw = np.asarray(ln_weight, dtype=np.float32)

    am = np.clip(np.abs(xf).max(axis=1), EPS, None).astype(np.float32)
    s = np.float32(127.0) / am
    qx = np.clip(np.rint(xf * s[:, None]), -128.0, 127.0).astype(np.float32)
    mc = (np.float32(1.0) / s).astype(np.float32)

    qxT = np.ascontiguousarray(qx.T).astype(ml_dtypes.bfloat16)
    bmc = (mc * mg).astype(np.float32)
    mcu = (mc * mu).astype(np.float32)
    meta = np.zeros(8, np.float32)
    meta[0] = md

    qgT = np.ascontiguousarray(qg.T)   # [H, I]
    quT = np.ascontiguousarray(qu.T)   # [H, I]
    qdT = np.ascontiguousarray(qd.T)   # [I, 2048]

    in_maps = []
    for r in range(N_CORES):
        c0 = r * ISH
        in_maps.append({
            "qxT": qxT,
            "wg": np.ascontiguousarray(qgT[:, c0:c0 + ISH])
                    .astype(ml_dtypes.float8_e4m3fn),
            "wu": np.ascontiguousarray(quT[:, c0:c0 + ISH])
                    .astype(ml_dtypes.float8_e4m3fn),
            "wd": np.ascontiguousarray(qdT[c0:c0 + ISH, :])
                    .astype(ml_dtypes.float8_e4m3fn),
            "lnw": np.ascontiguousarray(lnw[c0:c0 + ISH]),
            "bmc": bmc,
            "mcu": mcu,
            "meta": meta,
        })
    return in_maps


def _assemble(results):
    out = np.empty((T, 2048), dtype=np.float32)
    for r in range(N_CORES):
        yr = np.asarray(results[r]["y_out"]).astype(np.float32)
        for g in range(NG):
            t0 = g * TG + r * RPG
            out[t0:t0 + RPG] = yr[g * RPG:(g + 1) * RPG]
    return out.reshape(B, S, 2048)


def kernel(x, w_gate, w_up, w_down, ln_weight):
    from concourse import bass_utils

    nc = _get_nc()
    in_maps = _make_in_maps(x, w_gate, w_up, w_down, ln_weight)
    res = bass_utils.run_bass_kernel_spmd(nc, in_maps,
                                          core_ids=list(range(N_CORES)))
    return _assemble(res.results)
